# revision 53
# baseline (speedup 1.0000x reference)
"""AgentGNN v2.1 (2x CGConv + BN + residual + ReLU) on 8 TRN2 NeuronCores.

Self-contained: FULL inputs -> shard 8 samples/core -> Bass kernel -> FULL out.

Engine mapping per layer (per core: 8 samples, pairwise = 8x[128,64,64]):
  PE:  per-sample transposed projections A_c[t,p] = [alpha^T; beta^T] (x and
       centers+bias parts as matmuls), then P1 = A_c @ E with E a fixed 0/1
       indicator [128, 4096+64]: P1[p,(i,j)] = alpha[p,i]+beta[p,j]; the
       extra 64 cols give the diagonal alpha_i+beta_i. Normal-orientation
       matmuls for gamma/delta.
  ACT: sigmoid straight from PSUM chunks -> S16 (bf16); softplus as
       ln(1+exp(g)*exp(d)-factorized); samples processed in two groups of 4
       so sigmoid<->ln table sets load 5x/layer instead of 16x.
  DVE: P2 outer-mult u = exp(gamma_i)*exp(delta_j) (broadcast tt bf16),
       fused mult+prefix-scan custom op on (S16,T16) with segment-end
       writes, per-segment diffs -> row sums; BN stats (sum/sumsq);
       fused BN-apply+residual+relu.
  BN stats via TWO AllReduces/layer (one per sample-group: the first hides
  under the second group's compute, only the second's latency is exposed).
  All weights/inputs packed into 4 DMA blobs to dodge per-transfer latency.
"""

import numpy as np

N_SAMPLES = 64
N_AGENTS = 64
N = N_SAMPLES * N_AGENTS          # 4096
F = 128
EDIM = 2
BN_EPS = 1e-5
N_CORES = 8
S_PC = N_SAMPLES // N_CORES       # 8 samples per core
NODES_PC = S_PC * N_AGENTS        # 512 nodes per core
PAIR = N_AGENTS * N_AGENTS        # 4096 pairwise per sample
G_SZ = 4                          # samples per ACT-table group
WCOLS = 8 * F + N_AGENTS          # wblob: 8 weight mats | Ed
CCOLS = NODES_PC + 8 * F          # c3blob: c3 | 8 small mats

_CACHE = {}


def _patch_ldw_opt():
    from concourse import bass_utils as BU

    if getattr(BU, "_ldw_patched", False):
        return
    orig = BU.run_command

    def rc(cmd, *a, **kw):
        if isinstance(cmd, list):
            cmd = ["--enable-ldw-opt=true" if c == "--enable-ldw-opt=false" else c
                   for c in cmd]
        return orig(cmd, *a, **kw)

    BU.run_command = rc
    BU._ldw_patched = True


def _register_custom_ops():
    import numpy as _np
    from concourse import dve_ops as D

    if getattr(D, "_agnn_ops", None):
        return D._agnn_ops
    from concourse.dve_spec import Spec, Src0, Src1, C0, C1, AluOp, scan, lower
    from concourse.dve_uop import DveOpSpec
    from concourse.dve_spec import relu as dve_relu

    def ref_mult_scan(in0, in1, s0, s1, imm2):
        prod = (in0.astype(_np.float32) * in1 - s0).astype(_np.float32)
        return _np.cumsum(prod.reshape(prod.shape[0], -1), 1).astype(
            _np.float32).reshape(in0.shape)

    def ref_diff_add(in0, in1, s0, s1, imm2):
        return (in0.astype(_np.float32) - in1 + s0).astype(_np.float32)

    def ref_bn_res(in0, in1, s0, s1, imm2):
        return _np.maximum(in0.astype(_np.float32) * s0 - s1 + in1, 0.0).astype(
            _np.float32)

    def make(name, spec, subdim):
        row = D._CUSTOM_DVE_ROW_BASE + len(D.OPS)
        D._SUB_OPCODE_FOR_NAME[name] = row
        shas = {}
        for ver in ("v3", "v4"):
            u = lower(spec, ver=ver)
            shas[ver] = DveOpSpec(name=name, opcode=row, uops=u, rd1_en=True).sha(ver)
        op = D.DveOp(name, spec, subdim=subdim, uops_sha=shas)
        D.OPS.append(op)
        D.CUSTOM_DVE_SPECS[name] = spec
        return op

    sc = Spec(body=scan(AluOp.ADD, Src0 * Src1 - C0), reference=ref_mult_scan)
    df = Spec(body=Src0 - Src1 + C0, reference=ref_diff_add)
    br = Spec(body=dve_relu(Src0 * C0 - C1 + Src1), reference=ref_bn_res)
    D._agnn_ops = (make("AGNN_MULT_CSCAN", sc, True),
                   make("AGNN_DIFF_ADD", df, False),
                   make("AGNN_BN_RES", br, False))
    return D._agnn_ops


def _patch_act_tables():
    from concourse import bacc, mybir, hw_specs

    if getattr(bacc, "_act_tables_patched", False):
        return
    AF = mybir.ActivationFunctionType
    orig = hw_specs.get_activation_tables

    def patched(arch):
        t = orig(arch)
        out = {}
        for name, s in t.items():
            s = set(s)
            if name == "exp_and_others":
                s.discard(AF.Exp)
            if name == "natural_log":
                s.discard(AF.Ln)
            out[name] = s
        return out

    bacc.get_activation_tables = patched
    bacc._act_tables_patched = True


def _build_nc():
    from concourse import bacc, mybir
    from concourse.tile import TileContext
    from concourse.tile_rust import add_dep_helper

    _patch_act_tables()
    OP_SCAN, OP_DIFF, OP_BNRES = _register_custom_ops()

    f32 = mybir.dt.float32
    bf16 = mybir.dt.bfloat16
    AF = mybir.ActivationFunctionType
    OP = mybir.AluOpType
    AX = mybir.AxisListType

    nc = bacc.Bacc(trn_type="TRN2", target_bir_lowering=False, debug=False,
                   num_devices=N_CORES)

    xT16 = nc.declare_dram_parameter("xT16", [F, NODES_PC], bf16, isOutput=False)
    wblob = nc.declare_dram_parameter("wblob", [F, WCOLS], bf16, isOutput=False)
    f8 = mybir.dt.float8e4
    Eblob = nc.declare_dram_parameter("Eblob", [F, PAIR], f8, isOutput=False)
    c3blob = nc.declare_dram_parameter("c3blob", [EDIM + 1, CCOLS], bf16, isOutput=False)
    gblob = nc.declare_dram_parameter("gblob", [F, 4], f32, isOutput=False)
    cpAT = nc.declare_dram_parameter("cpAT", [F, 2 * S_PC * F], bf16, isOutput=False)
    yT = nc.declare_dram_parameter("yT", [F, NODES_PC], f32, isOutput=True)

    cc_warm_in = nc.dram_tensor("cc_warm_in", [F, 2], f32)
    cc_warm_out = nc.dram_tensor("cc_warm_out", [F, 2], f32, addr_space="Shared")
    cc_in = {}
    cc_out = {}
    for l in (1, 2):
        for g in (0, 1):
            cc_in[(l, g)] = nc.dram_tensor(f"cc_in{l}{g}", [F, 2], f32)
            cc_out[(l, g)] = nc.dram_tensor(f"cc_out{l}{g}", [F, 2], f32,
                                            addr_space="Shared")

    with TileContext(nc) as tc:
        from contextlib import ExitStack
        with ExitStack() as ctx:
            io = ctx.enter_context(tc.tile_pool(name="io", bufs=1))
            node = ctx.enter_context(tc.tile_pool(name="node", bufs=1))
            pair = ctx.enter_context(tc.tile_pool(name="pair", bufs=1))
            psum = ctx.enter_context(tc.tile_pool(name="psum", bufs=1, space="PSUM"))
            small = ctx.enter_context(tc.tile_pool(name="small", bufs=1))

            # order: small tensors the first matmuls/exps need come first;
            # the big Eblob (1MB) and cpa stream in behind them.
            xt16 = io.tile([F, NODES_PC], bf16, tag="xt16")
            nc.sync.dma_start(xt16[:], xT16.ap()[:, :])
            c3b = io.tile([EDIM + 1, CCOLS], bf16, tag="c3b")
            nc.sync.dma_start(c3b[:], c3blob.ap()[:, :])
            wb = io.tile([F, WCOLS], bf16, tag="wb")
            nc.sync.dma_start(wb[:], wblob.ap()[:, :])
            gb = io.tile([F, 4], f32, tag="gb")
            nc.sync.dma_start(gb[:], gblob.ap()[:, :])
            cpa = io.tile([F, 2 * S_PC * F], bf16, tag="cpa")
            nc.sync.dma_start(cpa[:], cpAT.ap()[:, :])
            eb = io.tile([F, PAIR], f8, tag="eb")
            nc.sync.dma_start(eb[:], Eblob.ap()[:, :])

            e16 = eb[:][:, :]
            ed16 = wb[:][:, 8 * F:8 * F + N_AGENTS]
            wslice = {}
            for li, l in enumerate((1, 2)):
                for wi, n in enumerate(("WaT", "WbT", "VaT", "VbT")):
                    c0 = (li * 4 + wi) * F
                    wslice[f"{n}{l}"] = wb[:][:, c0:c0 + F]
                for wi, n in enumerate(("Wc3a", "Wc3b", "Vc3g", "Vc3d")):
                    c0 = NODES_PC + (li * 4 + wi) * F
                    wslice[f"{n}{l}"] = c3b[:][:, c0:c0 + F]
            c3t = c3b[:][:, 0:NODES_PC]
            gam = {1: gb[:][:, 0:1], 2: gb[:][:, 2:3]}
            bet = {1: gb[:][:, 1:2], 2: gb[:][:, 3:4]}

            eps_t = small.tile([F, 1], f32, tag="eps")
            nc.vector.memset(eps_t[:], BN_EPS)
            zero1 = small.tile([F, 1], f32, tag="zero1")
            nc.vector.memset(zero1[:], 0.0)

            # one warm AR, triggered immediately: its doorbell absorbs the
            # ~60us CC-stream startup latency during the DMA-in/head phase
            nc.gpsimd.collective_compute(
                "AllReduce", mybir.AluOpType.add,
                replica_groups=[list(range(N_CORES))],
                ins=[cc_warm_in.ap().opt()], outs=[cc_warm_out.ap().opt()])

            act_chain = []

            def act(*args, **kw):
                i = nc.scalar.activation(*args, **kw)
                if act_chain:
                    add_dep_helper(i.ins, act_chain[-1].ins, reason="act order")
                act_chain.append(i)
                return i

            # persistent tiles
            Sall = pair.tile([F, S_PC * PAIR], bf16, tag="Sall")
            Uall = pair.tile([F, S_PC * PAIR], bf16, tag="Uall")
            S16 = [Sall[:][:, c * PAIR:(c + 1) * PAIR] for c in range(S_PC)]
            U16 = [Uall[:][:, c * PAIR:(c + 1) * PAIR] for c in range(S_PC)]
            A16 = [node.tile([F, F], bf16, tag=f"A{c}", name=f"A16_{c}")
                   for c in range(2)]
            eg = node.tile([F, NODES_PC], bf16, tag="eg")
            edt = node.tile([F, NODES_PC], bf16, tag="edt")
            d1s = node.tile([F, NODES_PC], bf16, tag="d1s")
            d2s = node.tile([F, NODES_PC], bf16, tag="d2s")
            dm = node.tile([F, NODES_PC], bf16, tag="dm")
            agg = node.tile([F, NODES_PC], f32, tag="agg")
            sq = node.tile([F, NODES_PC], f32, tag="sq")
            pref = node.tile([F, S_PC, N_AGENTS + 1], f32, tag="pref")

            psA = [psum.tile([F, 192], f32, tag=f"psA{b}", name=f"psA{b}")
                   for b in range(2)]
            psS = [psum.tile([F, 1024], f32, tag=f"psS{b}", name=f"psS{b}")
                   for b in range(3)]

            def pe_warm(n, bufs=(0, 1, 2)):
                # keep PE_HAM's activity window busy so the clock stays 8/8
                for i in range(n):
                    pst = psS[bufs[i % len(bufs)]]
                    nc.tensor.matmul(pst[:][:, 0:512], wslice["WaT1"],
                                     wb[:][:, 0:512], start=True, stop=True)

            def layer(l, x_in, x_out):
                # gamma/delta normal-orientation matmuls into psS[0] + exps
                nc.tensor.matmul(psS[0][:][:, 0:512], wslice[f"VaT{l}"], x_in,
                                 start=True, stop=False)
                nc.tensor.matmul(psS[0][:][:, 0:512], wslice[f"Vc3g{l}"], c3t,
                                 start=False, stop=True)
                nc.tensor.matmul(psS[0][:][:, 512:1024], wslice[f"VbT{l}"], x_in,
                                 start=True, stop=False)
                nc.tensor.matmul(psS[0][:][:, 512:1024], wslice[f"Vc3d{l}"], c3t,
                                 start=False, stop=True)
                act(eg[:], psS[0][:][:, 0:512], AF.Exp)
                act(edt[:], psS[0][:][:, 512:1024], AF.Exp)
                # dm = eg*ed (node level, feeds d2s = ln1p) — Pool engine
                dm_op = nc.gpsimd.tensor_tensor(dm[:], eg[:], edt[:], op=OP.mult)
                # all 8 outer-product u-mults on the (otherwise idle) Pool
                # engine, emitted early so U16[c] is ready before ACT's Ln
                def u_mult(c, eng):
                    sl_ = slice(c * N_AGENTS, (c + 1) * N_AGENTS)
                    g_bc = eg[:, sl_].broadcast_to([F, N_AGENTS, N_AGENTS])
                    d_bc = edt[:, sl_].rearrange("p (o j) -> p o j", o=1) \
                        .broadcast_to([F, N_AGENTS, N_AGENTS])
                    return eng.tensor_tensor(
                        U16[c].rearrange("p (i j) -> p i j", i=N_AGENTS),
                        g_bc, d_bc, op=OP.mult)

                u_last = dm_op

                s2 = {g: small.tile([F, 2], f32, tag=f"s2_{l}{g}",
                                    name=f"s2_{l}{g}") for g in (0, 1)}
                gst = {g: small.tile([F, 2], f32, tag=f"gst_{l}{g}",
                                     name=f"gst_{l}{g}") for g in (0, 1)}
                din = {}

                nc.vector.memset(pref[:, :, 0:1], 0.0)

                def sample_block(c, first, last):
                    # PE projections + E-matmul chunks + sigmoids for sample c
                    sl = slice(c * N_AGENTS, (c + 1) * N_AGENTS)
                    pa = psA[c % 2]
                    cp0 = (l - 1) * S_PC * F + c * F
                    nc.tensor.matmul(pa[:][0:64, 0:128], x_in[:, sl],
                                     wslice[f"WaT{l}"], start=True, stop=True)
                    nc.tensor.matmul(pa[:][64:128, 0:128], x_in[:, sl],
                                     wslice[f"WbT{l}"], start=True, stop=True)
                    a16 = A16[c % 2]
                    nc.vector.tensor_tensor(a16[:], pa[:][:, 0:128],
                                            cpa[:][:, cp0:cp0 + F], op=OP.add)
                    # DVE work for the PREVIOUS sample goes here (after the
                    # add, so E-matmuls never wait a long DVE op for a16):
                    # group 0 interleaves u-mults, group 1 interleaves scans
                    if not first:
                        if c <= G_SZ:
                            u_mult(c - 1, nc.vector)
                        else:
                            scan(c - 1)
                    for q in range(4):
                        pst = psS[q % 3]
                        for h in range(2):
                            col = q * 1024 + h * 512
                            nc.tensor.matmul(
                                pst[:][:, h * 512:(h + 1) * 512], a16[:],
                                e16[:, col:col + 512], start=True, stop=True)
                        act(S16[c][:, q * 1024:(q + 1) * 1024], pst[:],
                            AF.Sigmoid)
                    nc.tensor.matmul(pa[:][:, 128:192], a16[:], ed16,
                                     start=True, stop=True)
                    act(d1s[:][:, sl], pa[:][:, 128:192], AF.Sigmoid)
                    if last:
                        if c < G_SZ:
                            u_mult(c, nc.vector)
                        else:
                            scan(c)

                def scan(c):
                    scan_out = pref[:, c, 1:1 + N_AGENTS] \
                        .rearrange("p (i o) -> p i o", o=1) \
                        .broadcast_to([F, N_AGENTS, N_AGENTS])
                    nc.vector._custom_dve(
                        OP_SCAN, out=scan_out,
                        in0=S16[c].rearrange("p (i j) -> p i j", i=N_AGENTS),
                        in1=U16[c].rearrange("p (i j) -> p i j", i=N_AGENTS),
                        s0=zero1[:, 0:1])

                def group_stats(g):
                    gsl = slice(g * G_SZ * N_AGENTS, (g + 1) * G_SZ * N_AGENTS)
                    gs = slice(g * G_SZ, (g + 1) * G_SZ)
                    nc.vector._custom_dve(
                        OP_DIFF,
                        out=agg[:, gsl].rearrange("p (c i) -> p c i", c=G_SZ),
                        in0=pref[:, gs, 1:1 + N_AGENTS],
                        in1=pref[:, gs, 0:N_AGENTS],
                        s0=zero1[:, 0:1])
                    return gsl

                def stats_and_ar(g):
                    gsl = group_stats(g)
                    # self-msg subtract + BN partial stats for this group (DVE)
                    nc.vector.tensor_tensor(dm[:, gsl], d1s[:, gsl], d2s[:, gsl],
                                            op=OP.mult)
                    nc.vector.tensor_tensor(agg[:, gsl], agg[:, gsl], dm[:, gsl],
                                            op=OP.subtract)
                    nc.vector.tensor_reduce(s2[g][:, 0:1], agg[:, gsl],
                                            axis=AX.X, op=OP.add)
                    nc.vector.tensor_tensor(sq[:, gsl], agg[:, gsl], agg[:, gsl],
                                            op=OP.mult)
                    nc.vector.tensor_reduce(s2[g][:, 1:2], sq[:, gsl],
                                            axis=AX.X, op=OP.add)
                    dsum = nc.sync.dma_start(cc_in[(l, g)].ap()[:, :], s2[g][:])
                    ar = nc.gpsimd.collective_compute(
                        "AllReduce", mybir.AluOpType.add,
                        replica_groups=[list(range(N_CORES))],
                        ins=[cc_in[(l, g)].ap().opt()],
                        outs=[cc_out[(l, g)].ap().opt()])
                    add_dep_helper(ar.ins, dsum.ins, reason="cc reads cc_in")
                    # keep the AR's Pool-queue doorbell BEHIND all u-mults
                    # (head-of-line blocking otherwise stalls them on CC)
                    add_dep_helper(ar.ins, u_last.ins, reason="doorbell after u-mults")
                    din[g] = nc.sync.dma_start(gst[g][:], cc_out[(l, g)].ap()[:, :])
                    add_dep_helper(din[g].ins, ar.ins, reason="dma reads cc_out")


                # ---- phase 1: group-0 samples (sigmoid table), u-mults 0-3
                # interleaved on DVE; u4-7 emitted right after (they only
                # need eg/edt and must be done before the batched Ln section)
                for c in range(G_SZ):
                    sample_block(c, c == 0, c == G_SZ - 1)
                for c in range(G_SZ, S_PC):
                    u_mult(c, nc.vector)
                # ---- phase 2: ONE ln-table section for d2s + ALL 8 U16 Lns;
                # group-0 scans interleave behind their Lns on DVE
                act(d2s[:], dm[:], AF.Ln, bias=1.0)
                for c in range(G_SZ):
                    act(U16[c], U16[c], AF.Ln, bias=1.0)
                    scan(c)
                stats_and_ar(0)
                for c in range(G_SZ, S_PC):
                    act(U16[c], U16[c], AF.Ln, bias=1.0)
                # ---- phase 3: group-1 samples (sigmoid table) with their
                # scans interleaved (U16 already Ln'd); then stats
                for c in range(G_SZ, S_PC):
                    sample_block(c, c == G_SZ, c == S_PC - 1)

                stats_and_ar(1)

                # ---- BN apply + residual + relu ----
                gall = small.tile([F, 2], f32, tag=f"gall_{l}", name=f"gall_{l}")
                nc.vector.tensor_tensor(gall[:], gst[0][:], gst[1][:], op=OP.add)
                me2 = small.tile([F, 2], f32, tag=f"me2_{l}", name=f"me2_{l}")
                nc.vector.tensor_scalar(me2[:], gall[:], 1.0 / N, None, op0=OP.mult)
                mean, ex2 = me2[:, 0:1], me2[:, 1:2]
                var = small.tile([F, 1], f32, tag=f"var_{l}", name=f"var_{l}")
                nc.vector.tensor_tensor(var[:], mean, mean, op=OP.mult)
                nc.vector.tensor_tensor(var[:], ex2, var[:], op=OP.subtract)
                lnv = small.tile([F, 1], f32, tag=f"lnv_{l}", name=f"lnv_{l}")
                act(lnv[:], var[:], AF.Ln, bias=eps_t[:])
                rstd = small.tile([F, 1], f32, tag=f"rstd_{l}", name=f"rstd_{l}")
                act(rstd[:], lnv[:], AF.Exp, bias=0.0, scale=-0.5)
                scal = small.tile([F, 1], f32, tag=f"scal_{l}", name=f"scal_{l}")
                nc.vector.tensor_tensor(scal[:], rstd[:], gam[l], op=OP.mult)
                shneg = small.tile([F, 1], f32, tag=f"shneg_{l}", name=f"shneg_{l}")
                nc.vector.tensor_scalar(shneg[:], mean, scal[:, 0:1], bet[l],
                                        op0=OP.mult, op1=OP.subtract)
                nc.vector._custom_dve(OP_BNRES, out=x_out, in0=agg[:],
                                      in1=x_in, s0=scal[:, 0:1], s1=shneg[:, 0:1])

            x1 = io.tile([F, NODES_PC], bf16, tag="x1")
            layer(1, xt16[:], x1[:])
            y32 = io.tile([F, NODES_PC], f32, tag="y32")
            layer(2, x1[:], y32[:])
            nc.sync.dma_start(yT.ap()[:, 0:256], y32[:][:, 0:256])
            nc.sync.dma_start(yT.ap()[:, 256:NODES_PC], y32[:][:, 256:NODES_PC])

    nc.compile()
    return nc


def _get_nc():
    if "nc" not in _CACHE:
        _CACHE["nc"] = _build_nc()
    return _CACHE["nc"]


def _canonical_edge_ok(src, dst):
    idx = np.arange(N_AGENTS)
    rows = np.repeat(idx, N_AGENTS)
    cols = np.tile(idx, N_AGENTS)
    m = rows != cols
    rows, cols = rows[m], cols[m]
    offs = (np.arange(N_SAMPLES) * N_AGENTS)[:, None]
    csrc = (rows[None, :] + offs).ravel().astype(np.int64)
    cdst = (cols[None, :] + offs).ravel().astype(np.int64)
    if src.shape != csrc.shape:
        return False
    key = np.sort(src.astype(np.int64) * N + dst.astype(np.int64))
    ckey = np.sort(csrc * N + cdst)
    return bool(np.array_equal(key, ckey))


def _numpy_fallback(gnn_in, centers, src, dst, Ws_all):
    def sig(x):
        return 1.0 / (1.0 + np.exp(-x))

    def sp(x):
        return np.log1p(np.exp(-np.abs(x))) + np.maximum(x, 0.0)

    x = gnn_in.astype(np.float64)
    e = (centers[dst] - centers[src]).astype(np.float64)
    for (Wf, bf, Wsm, bs, g, be) in Ws_all:
        z = np.concatenate([x[dst], x[src], e], axis=-1)
        msg = sig(z @ Wf.T + bf) * sp(z @ Wsm.T + bs)
        agg = np.zeros_like(x)
        np.add.at(agg, dst, msg)
        mean = agg.mean(0)
        var = agg.var(0)
        agg = (agg - mean) / np.sqrt(var + BN_EPS) * g + be
        x = np.maximum(agg + x, 0.0)
    return x.astype(np.float32)


def _host_weights(Wf, bf, Ws, bs):
    WaT = np.ascontiguousarray(Wf[:, :F].T)
    WbT = np.ascontiguousarray(Wf[:, F:2 * F].T)
    Wc = Wf[:, 2 * F:2 * F + EDIM].T
    z = np.zeros((1, F), np.float32)
    Wc3a = np.concatenate([Wc, bf[None, :]], 0)
    Wc3b = np.concatenate([-Wc, z], 0)
    VaT = np.ascontiguousarray(Ws[:, :F].T)
    VbT = np.ascontiguousarray(Ws[:, F:2 * F].T)
    Vc = Ws[:, 2 * F:2 * F + EDIM].T
    Vc3g = np.concatenate([Vc, bs[None, :]], 0)
    Vc3d = np.concatenate([-Vc, z], 0)
    return WaT, WbT, VaT, VbT, Wc3a, Wc3b, Vc3g, Vc3d


def _build_E16():
    E = np.zeros((F, N_AGENTS, N_AGENTS), np.float32)
    for t in range(N_AGENTS):
        E[t, t, :] = 1.0
        E[N_AGENTS + t, :, t] = 1.0
    E = E.reshape(F, PAIR)
    Ed = np.zeros((F, N_AGENTS), np.float32)
    for t in range(N_AGENTS):
        Ed[t, t] = 1.0
        Ed[N_AGENTS + t, t] = 1.0
    return E, Ed


def kernel(gnn_in, centers, src, dst,
           Wf1, bf1, Ws1, bs1, g1, be1,
           Wf2, bf2, Ws2, bs2, g2, be2,
           _trace=False, _tmpdir=None):
    import ml_dtypes
    b = ml_dtypes.bfloat16
    gnn_in = np.ascontiguousarray(np.asarray(gnn_in, np.float32))
    centers = np.ascontiguousarray(np.asarray(centers, np.float32))
    src = np.asarray(src, np.int32)
    dst = np.asarray(dst, np.int32)
    args = [np.asarray(a, np.float32) for a in
            (Wf1, bf1, Ws1, bs1, g1, be1, Wf2, bf2, Ws2, bs2, g2, be2)]
    (Wf1, bf1, Ws1, bs1, g1, be1, Wf2, bf2, Ws2, bs2, g2, be2) = args

    if not _canonical_edge_ok(src, dst):
        import sys
        print("kernel.py: edge index is not block-fully-connected; numpy fallback",
              file=sys.stderr)
        return _numpy_fallback(gnn_in, centers, src, dst,
                               [(Wf1, bf1, Ws1, bs1, g1, be1),
                                (Wf2, bf2, Ws2, bs2, g2, be2)])

    from concourse import bass_utils

    nc = _get_nc()

    E, Ed = _build_E16()
    w1 = _host_weights(Wf1, bf1, Ws1, bs1)
    w2 = _host_weights(Wf2, bf2, Ws2, bs2)
    wb = np.concatenate(list(w1[:4]) + list(w2[:4]) + [Ed], 1)
    wmap = {
        "wblob": np.ascontiguousarray(wb).astype(b),
        "Eblob": np.ascontiguousarray(E).astype(ml_dtypes.float8_e4m3),
        "gblob": np.ascontiguousarray(
            np.stack([g1, be1, g2, be2], 1).astype(np.float32)),
    }

    in_maps = []
    for k in range(N_CORES):
        sl = slice(k * NODES_PC, (k + 1) * NODES_PC)
        m = dict(wmap)
        m["xT16"] = np.ascontiguousarray(gnn_in[sl].T).astype(b)
        c3k = np.concatenate([centers[sl].T, np.ones((1, NODES_PC), np.float32)], 0)
        cb = np.concatenate([c3k] + list(w1[4:]) + list(w2[4:]), 1)
        m["c3blob"] = np.ascontiguousarray(cb).astype(b)
        c3b16 = c3k.astype(b).astype(np.float32)
        cps = []
        for w in (w1, w2):
            Wc3a = w[4].astype(b).astype(np.float32)
            Wc3b = w[5].astype(b).astype(np.float32)
            for c in range(S_PC):
                csl = c3b16[:, c * N_AGENTS:(c + 1) * N_AGENTS]
                cps.append(np.concatenate([csl.T @ Wc3a, csl.T @ Wc3b], 0))
        m["cpAT"] = np.ascontiguousarray(np.concatenate(cps, 1)).astype(b)
        in_maps.append(m)

    kw = {}
    if _trace:
        kw = dict(trace=True, tmpdir=_tmpdir)
    res = bass_utils.run_bass_kernel_spmd(nc, in_maps, core_ids=list(range(N_CORES)), **kw)

    out = np.empty((N, F), np.float32)
    for k in range(N_CORES):
        out[k * NODES_PC:(k + 1) * NODES_PC] = res.results[k]["yT"].T
    if _trace:
        _CACHE["last_res"] = res
    return out



# revision 55
# speedup vs baseline: 1.0099x; 1.0099x over previous
"""AgentGNN v2.1 (2x CGConv + BN + residual + ReLU) on 8 TRN2 NeuronCores.

Self-contained: FULL inputs -> shard 8 samples/core -> Bass kernel -> FULL out.

Engine mapping per layer (per core: 8 samples, pairwise = 8x[128,64,64]):
  PE:  per-sample transposed projections A_c[t,p] = [alpha^T; beta^T] (x and
       centers+bias parts as matmuls), then P1 = A_c @ E with E a fixed 0/1
       indicator [128, 4096+64]: P1[p,(i,j)] = alpha[p,i]+beta[p,j]; the
       extra 64 cols give the diagonal alpha_i+beta_i. Normal-orientation
       matmuls for gamma/delta.
  ACT: sigmoid straight from PSUM chunks -> S16 (bf16); softplus as
       ln(1+exp(g)*exp(d)-factorized); samples processed in two groups of 4
       so sigmoid<->ln table sets load 5x/layer instead of 16x.
  DVE: P2 outer-mult u = exp(gamma_i)*exp(delta_j) (broadcast tt bf16),
       fused mult+prefix-scan custom op on (S16,T16) with segment-end
       writes, per-segment diffs -> row sums; BN stats (sum/sumsq);
       fused BN-apply+residual+relu.
  BN stats via TWO AllReduces/layer (one per sample-group: the first hides
  under the second group's compute, only the second's latency is exposed).
  All weights/inputs packed into 4 DMA blobs to dodge per-transfer latency.
"""

import numpy as np

N_SAMPLES = 64
N_AGENTS = 64
N = N_SAMPLES * N_AGENTS          # 4096
F = 128
EDIM = 2
BN_EPS = 1e-5
N_CORES = 8
S_PC = N_SAMPLES // N_CORES       # 8 samples per core
NODES_PC = S_PC * N_AGENTS        # 512 nodes per core
PAIR = N_AGENTS * N_AGENTS        # 4096 pairwise per sample
G_SZ = 4                          # samples per ACT-table group
WCOLS = 8 * F + N_AGENTS          # wblob: 8 weight mats | Ed
CCOLS = NODES_PC + 8 * F          # c3blob: c3 | 8 small mats

_CACHE = {}


def _patch_ldw_opt():
    from concourse import bass_utils as BU

    if getattr(BU, "_ldw_patched", False):
        return
    orig = BU.run_command

    def rc(cmd, *a, **kw):
        if isinstance(cmd, list):
            cmd = ["--enable-ldw-opt=true" if c == "--enable-ldw-opt=false" else c
                   for c in cmd]
        return orig(cmd, *a, **kw)

    BU.run_command = rc
    BU._ldw_patched = True


def _register_custom_ops():
    import numpy as _np
    from concourse import dve_ops as D

    if getattr(D, "_agnn_ops", None):
        return D._agnn_ops
    from concourse.dve_spec import Spec, Src0, Src1, C0, C1, AluOp, scan, lower
    from concourse.dve_uop import DveOpSpec
    from concourse.dve_spec import relu as dve_relu

    def ref_mult_scan(in0, in1, s0, s1, imm2):
        prod = (in0.astype(_np.float32) * in1 - s0).astype(_np.float32)
        return _np.cumsum(prod.reshape(prod.shape[0], -1), 1).astype(
            _np.float32).reshape(in0.shape)

    def ref_diff_add(in0, in1, s0, s1, imm2):
        return (in0.astype(_np.float32) - in1 + s0).astype(_np.float32)

    def ref_bn_res(in0, in1, s0, s1, imm2):
        return _np.maximum(in0.astype(_np.float32) * s0 - s1 + in1, 0.0).astype(
            _np.float32)

    def make(name, spec, subdim):
        row = D._CUSTOM_DVE_ROW_BASE + len(D.OPS)
        D._SUB_OPCODE_FOR_NAME[name] = row
        shas = {}
        for ver in ("v3", "v4"):
            u = lower(spec, ver=ver)
            shas[ver] = DveOpSpec(name=name, opcode=row, uops=u, rd1_en=True).sha(ver)
        op = D.DveOp(name, spec, subdim=subdim, uops_sha=shas)
        D.OPS.append(op)
        D.CUSTOM_DVE_SPECS[name] = spec
        return op

    sc = Spec(body=scan(AluOp.ADD, Src0 * Src1 - C0), reference=ref_mult_scan)
    df = Spec(body=Src0 - Src1 + C0, reference=ref_diff_add)
    br = Spec(body=dve_relu(Src0 * C0 - C1 + Src1), reference=ref_bn_res)
    D._agnn_ops = (make("AGNN_MULT_CSCAN", sc, True),
                   make("AGNN_DIFF_ADD", df, False),
                   make("AGNN_BN_RES", br, False))
    return D._agnn_ops


def _patch_act_tables():
    from concourse import bacc, mybir, hw_specs

    if getattr(bacc, "_act_tables_patched", False):
        return
    AF = mybir.ActivationFunctionType
    orig = hw_specs.get_activation_tables

    def patched(arch):
        t = orig(arch)
        out = {}
        for name, s in t.items():
            s = set(s)
            if name == "exp_and_others":
                s.discard(AF.Exp)
            if name == "natural_log":
                s.discard(AF.Ln)
            out[name] = s
        return out

    bacc.get_activation_tables = patched
    bacc._act_tables_patched = True


def _build_nc():
    from concourse import bacc, mybir
    from concourse.tile import TileContext
    from concourse.tile_rust import add_dep_helper

    _patch_act_tables()
    OP_SCAN, OP_DIFF, OP_BNRES = _register_custom_ops()

    f32 = mybir.dt.float32
    bf16 = mybir.dt.bfloat16
    AF = mybir.ActivationFunctionType
    OP = mybir.AluOpType
    AX = mybir.AxisListType

    nc = bacc.Bacc(trn_type="TRN2", target_bir_lowering=False, debug=False,
                   num_devices=N_CORES)

    xT16 = nc.declare_dram_parameter("xT16", [F, NODES_PC], bf16, isOutput=False)
    wblob = nc.declare_dram_parameter("wblob", [F, WCOLS], bf16, isOutput=False)
    Eblob = nc.declare_dram_parameter("Eblob", [F, PAIR], bf16, isOutput=False)
    c3blob = nc.declare_dram_parameter("c3blob", [EDIM + 1, CCOLS], bf16, isOutput=False)
    gblob = nc.declare_dram_parameter("gblob", [F, 4], f32, isOutput=False)
    cpAT = nc.declare_dram_parameter("cpAT", [F, 2 * S_PC * F], bf16, isOutput=False)
    yT = nc.declare_dram_parameter("yT", [F, NODES_PC], f32, isOutput=True)

    cc_warm_in = nc.dram_tensor("cc_warm_in", [F, 2], f32)
    cc_warm_out = nc.dram_tensor("cc_warm_out", [F, 2], f32, addr_space="Shared")
    cc_in = {}
    cc_out = {}
    for l in (1, 2):
        for g in (0, 1):
            cc_in[(l, g)] = nc.dram_tensor(f"cc_in{l}{g}", [F, 2], f32)
            cc_out[(l, g)] = nc.dram_tensor(f"cc_out{l}{g}", [F, 2], f32,
                                            addr_space="Shared")

    with TileContext(nc) as tc:
        from contextlib import ExitStack
        with ExitStack() as ctx:
            io = ctx.enter_context(tc.tile_pool(name="io", bufs=1))
            node = ctx.enter_context(tc.tile_pool(name="node", bufs=1))
            pair = ctx.enter_context(tc.tile_pool(name="pair", bufs=1))
            psum = ctx.enter_context(tc.tile_pool(name="psum", bufs=1, space="PSUM"))
            small = ctx.enter_context(tc.tile_pool(name="small", bufs=1))

            # order: small tensors the first matmuls/exps need come first;
            # the big Eblob (1MB) and cpa stream in behind them.
            xt16 = io.tile([F, NODES_PC], bf16, tag="xt16")
            nc.sync.dma_start(xt16[:], xT16.ap()[:, :])
            c3b = io.tile([EDIM + 1, CCOLS], bf16, tag="c3b")
            nc.sync.dma_start(c3b[:], c3blob.ap()[:, :])
            wb = io.tile([F, WCOLS], bf16, tag="wb")
            nc.sync.dma_start(wb[:], wblob.ap()[:, :])
            gb = io.tile([F, 4], f32, tag="gb")
            nc.sync.dma_start(gb[:], gblob.ap()[:, :])
            cpa = io.tile([F, 2 * S_PC * F], bf16, tag="cpa")
            nc.sync.dma_start(cpa[:], cpAT.ap()[:, :])
            eb = io.tile([F, PAIR], bf16, tag="eb")
            nc.sync.dma_start(eb[:], Eblob.ap()[:, :])

            e16 = eb[:][:, :]
            ed16 = wb[:][:, 8 * F:8 * F + N_AGENTS]
            wslice = {}
            for li, l in enumerate((1, 2)):
                for wi, n in enumerate(("WaT", "WbT", "VaT", "VbT")):
                    c0 = (li * 4 + wi) * F
                    wslice[f"{n}{l}"] = wb[:][:, c0:c0 + F]
                for wi, n in enumerate(("Wc3a", "Wc3b", "Vc3g", "Vc3d")):
                    c0 = NODES_PC + (li * 4 + wi) * F
                    wslice[f"{n}{l}"] = c3b[:][:, c0:c0 + F]
            c3t = c3b[:][:, 0:NODES_PC]
            gam = {1: gb[:][:, 0:1], 2: gb[:][:, 2:3]}
            bet = {1: gb[:][:, 1:2], 2: gb[:][:, 3:4]}

            eps_t = small.tile([F, 1], f32, tag="eps")
            nc.vector.memset(eps_t[:], BN_EPS)
            zero1 = small.tile([F, 1], f32, tag="zero1")
            nc.vector.memset(zero1[:], 0.0)

            # one warm AR, triggered immediately: its doorbell absorbs the
            # ~60us CC-stream startup latency during the DMA-in/head phase
            nc.gpsimd.collective_compute(
                "AllReduce", mybir.AluOpType.add,
                replica_groups=[list(range(N_CORES))],
                ins=[cc_warm_in.ap().opt()], outs=[cc_warm_out.ap().opt()])

            act_chain = []

            def act(*args, **kw):
                i = nc.scalar.activation(*args, **kw)
                if act_chain:
                    add_dep_helper(i.ins, act_chain[-1].ins, reason="act order")
                act_chain.append(i)
                return i

            # persistent tiles
            Sall = pair.tile([F, S_PC * PAIR], bf16, tag="Sall")
            Uall = pair.tile([F, S_PC * PAIR], bf16, tag="Uall")
            S16 = [Sall[:][:, c * PAIR:(c + 1) * PAIR] for c in range(S_PC)]
            U16 = [Uall[:][:, c * PAIR:(c + 1) * PAIR] for c in range(S_PC)]
            A16 = [node.tile([F, F], bf16, tag=f"A{c}", name=f"A16_{c}")
                   for c in range(2)]
            eg = node.tile([F, NODES_PC], bf16, tag="eg")
            edt = node.tile([F, NODES_PC], bf16, tag="edt")
            d1s = node.tile([F, NODES_PC], bf16, tag="d1s")
            d2s = node.tile([F, NODES_PC], bf16, tag="d2s")
            dm = node.tile([F, NODES_PC], bf16, tag="dm")
            agg = node.tile([F, NODES_PC], f32, tag="agg")
            sq = node.tile([F, NODES_PC], f32, tag="sq")
            pref = node.tile([F, S_PC, N_AGENTS + 1], f32, tag="pref")

            psA = [psum.tile([F, 192], f32, tag=f"psA{b}", name=f"psA{b}")
                   for b in range(2)]
            psS = [psum.tile([F, 1024], f32, tag=f"psS{b}", name=f"psS{b}")
                   for b in range(3)]

            def pe_warm(n, bufs=(0, 1, 2)):
                # keep PE_HAM's activity window busy so the clock stays 8/8
                for i in range(n):
                    pst = psS[bufs[i % len(bufs)]]
                    nc.tensor.matmul(pst[:][:, 0:512], wslice["WaT1"],
                                     wb[:][:, 0:512], start=True, stop=True)

            def layer(l, x_in, x_out):
                # gamma/delta normal-orientation matmuls into psS[0] + exps
                nc.tensor.matmul(psS[0][:][:, 0:512], wslice[f"VaT{l}"], x_in,
                                 start=True, stop=False)
                nc.tensor.matmul(psS[0][:][:, 0:512], wslice[f"Vc3g{l}"], c3t,
                                 start=False, stop=True)
                nc.tensor.matmul(psS[0][:][:, 512:1024], wslice[f"VbT{l}"], x_in,
                                 start=True, stop=False)
                nc.tensor.matmul(psS[0][:][:, 512:1024], wslice[f"Vc3d{l}"], c3t,
                                 start=False, stop=True)
                act(eg[:], psS[0][:][:, 0:512], AF.Exp)
                act(edt[:], psS[0][:][:, 512:1024], AF.Exp)
                # dm = eg*ed (node level, feeds d2s = ln1p) — Pool engine
                dm_op = nc.gpsimd.tensor_tensor(dm[:], eg[:], edt[:], op=OP.mult)
                # all 8 outer-product u-mults on the (otherwise idle) Pool
                # engine, emitted early so U16[c] is ready before ACT's Ln
                def u_mult(c, eng):
                    sl_ = slice(c * N_AGENTS, (c + 1) * N_AGENTS)
                    g_bc = eg[:, sl_].broadcast_to([F, N_AGENTS, N_AGENTS])
                    d_bc = edt[:, sl_].rearrange("p (o j) -> p o j", o=1) \
                        .broadcast_to([F, N_AGENTS, N_AGENTS])
                    return eng.tensor_tensor(
                        U16[c].rearrange("p (i j) -> p i j", i=N_AGENTS),
                        g_bc, d_bc, op=OP.mult)

                u_last = dm_op

                s2 = {g: small.tile([F, 2], f32, tag=f"s2_{l}{g}",
                                    name=f"s2_{l}{g}") for g in (0, 1)}
                gst = {g: small.tile([F, 2], f32, tag=f"gst_{l}{g}",
                                     name=f"gst_{l}{g}") for g in (0, 1)}
                din = {}

                nc.vector.memset(pref[:, :, 0:1], 0.0)

                def sample_block(c, first, last):
                    # PE projections + E-matmul chunks + sigmoids for sample c
                    sl = slice(c * N_AGENTS, (c + 1) * N_AGENTS)
                    pa = psA[c % 2]
                    cp0 = (l - 1) * S_PC * F + c * F
                    nc.tensor.matmul(pa[:][0:64, 0:128], x_in[:, sl],
                                     wslice[f"WaT{l}"], start=True, stop=True)
                    nc.tensor.matmul(pa[:][64:128, 0:128], x_in[:, sl],
                                     wslice[f"WbT{l}"], start=True, stop=True)
                    a16 = A16[c % 2]
                    nc.vector.tensor_tensor(a16[:], pa[:][:, 0:128],
                                            cpa[:][:, cp0:cp0 + F], op=OP.add)
                    # DVE work for the PREVIOUS sample goes here (after the
                    # add, so E-matmuls never wait a long DVE op for a16):
                    # group 0 interleaves u-mults, group 1 interleaves scans
                    if not first:
                        if c <= G_SZ:
                            u_mult(c - 1, nc.vector)
                        else:
                            scan(c - 1)
                    for q in range(4):
                        pst = psS[q % 3]
                        for h in range(2):
                            col = q * 1024 + h * 512
                            nc.tensor.matmul(
                                pst[:][:, h * 512:(h + 1) * 512], a16[:],
                                e16[:, col:col + 512], start=True, stop=True)
                        act(S16[c][:, q * 1024:(q + 1) * 1024], pst[:],
                            AF.Sigmoid)
                    nc.tensor.matmul(pa[:][:, 128:192], a16[:], ed16,
                                     start=True, stop=True)
                    act(d1s[:][:, sl], pa[:][:, 128:192], AF.Sigmoid)
                    if last:
                        if c < G_SZ:
                            u_mult(c, nc.vector)
                        else:
                            scan(c)

                def scan(c):
                    scan_out = pref[:, c, 1:1 + N_AGENTS] \
                        .rearrange("p (i o) -> p i o", o=1) \
                        .broadcast_to([F, N_AGENTS, N_AGENTS])
                    nc.vector._custom_dve(
                        OP_SCAN, out=scan_out,
                        in0=S16[c].rearrange("p (i j) -> p i j", i=N_AGENTS),
                        in1=U16[c].rearrange("p (i j) -> p i j", i=N_AGENTS),
                        s0=zero1[:, 0:1])

                def group_stats(g):
                    gsl = slice(g * G_SZ * N_AGENTS, (g + 1) * G_SZ * N_AGENTS)
                    gs = slice(g * G_SZ, (g + 1) * G_SZ)
                    nc.vector._custom_dve(
                        OP_DIFF,
                        out=agg[:, gsl].rearrange("p (c i) -> p c i", c=G_SZ),
                        in0=pref[:, gs, 1:1 + N_AGENTS],
                        in1=pref[:, gs, 0:N_AGENTS],
                        s0=zero1[:, 0:1])
                    return gsl

                def stats_and_ar(g):
                    gsl = group_stats(g)
                    # self-msg subtract + BN partial stats for this group (DVE)
                    nc.vector.tensor_tensor(dm[:, gsl], d1s[:, gsl], d2s[:, gsl],
                                            op=OP.mult)
                    nc.vector.tensor_tensor(agg[:, gsl], agg[:, gsl], dm[:, gsl],
                                            op=OP.subtract)
                    nc.vector.tensor_reduce(s2[g][:, 0:1], agg[:, gsl],
                                            axis=AX.X, op=OP.add)
                    nc.vector.tensor_tensor(sq[:, gsl], agg[:, gsl], agg[:, gsl],
                                            op=OP.mult)
                    nc.vector.tensor_reduce(s2[g][:, 1:2], sq[:, gsl],
                                            axis=AX.X, op=OP.add)
                    dsum = nc.sync.dma_start(cc_in[(l, g)].ap()[:, :], s2[g][:])
                    ar = nc.gpsimd.collective_compute(
                        "AllReduce", mybir.AluOpType.add,
                        replica_groups=[list(range(N_CORES))],
                        ins=[cc_in[(l, g)].ap().opt()],
                        outs=[cc_out[(l, g)].ap().opt()])
                    add_dep_helper(ar.ins, dsum.ins, reason="cc reads cc_in")
                    # keep the AR's Pool-queue doorbell BEHIND all u-mults
                    # (head-of-line blocking otherwise stalls them on CC)
                    add_dep_helper(ar.ins, u_last.ins, reason="doorbell after u-mults")
                    din[g] = nc.sync.dma_start(gst[g][:], cc_out[(l, g)].ap()[:, :])
                    add_dep_helper(din[g].ins, ar.ins, reason="dma reads cc_out")


                # ---- phase 1: group-0 samples (sigmoid table), u-mults 0-3
                # interleaved on DVE; u4-7 emitted right after (they only
                # need eg/edt and must be done before the batched Ln section)
                for c in range(G_SZ):
                    sample_block(c, c == 0, c == G_SZ - 1)
                for c in range(G_SZ, S_PC):
                    u_mult(c, nc.vector)
                # ---- phase 2: ONE ln-table section for d2s + ALL 8 U16 Lns;
                # group-0 scans interleave behind their Lns on DVE
                act(d2s[:], dm[:], AF.Ln, bias=1.0)
                # Lns batched per sample-pair (contiguous in Uall): fewer
                # ACT instruction overheads on the phase-2 critical path
                for c in range(0, G_SZ, 2):
                    uu = Uall[:][:, c * PAIR:(c + 2) * PAIR]
                    act(uu, uu, AF.Ln, bias=1.0)
                    scan(c)
                    scan(c + 1)
                stats_and_ar(0)
                for c in range(G_SZ, S_PC, 2):
                    uu = Uall[:][:, c * PAIR:(c + 2) * PAIR]
                    act(uu, uu, AF.Ln, bias=1.0)
                # ---- phase 3: group-1 samples (sigmoid table) with their
                # scans interleaved (U16 already Ln'd); then stats
                for c in range(G_SZ, S_PC):
                    sample_block(c, c == G_SZ, c == S_PC - 1)

                stats_and_ar(1)

                # ---- BN apply + residual + relu ----
                gall = small.tile([F, 2], f32, tag=f"gall_{l}", name=f"gall_{l}")
                nc.vector.tensor_tensor(gall[:], gst[0][:], gst[1][:], op=OP.add)
                me2 = small.tile([F, 2], f32, tag=f"me2_{l}", name=f"me2_{l}")
                nc.vector.tensor_scalar(me2[:], gall[:], 1.0 / N, None, op0=OP.mult)
                mean, ex2 = me2[:, 0:1], me2[:, 1:2]
                var = small.tile([F, 1], f32, tag=f"var_{l}", name=f"var_{l}")
                nc.vector.tensor_tensor(var[:], mean, mean, op=OP.mult)
                nc.vector.tensor_tensor(var[:], ex2, var[:], op=OP.subtract)
                lnv = small.tile([F, 1], f32, tag=f"lnv_{l}", name=f"lnv_{l}")
                act(lnv[:], var[:], AF.Ln, bias=eps_t[:])
                rstd = small.tile([F, 1], f32, tag=f"rstd_{l}", name=f"rstd_{l}")
                act(rstd[:], lnv[:], AF.Exp, bias=0.0, scale=-0.5)
                scal = small.tile([F, 1], f32, tag=f"scal_{l}", name=f"scal_{l}")
                nc.vector.tensor_tensor(scal[:], rstd[:], gam[l], op=OP.mult)
                shneg = small.tile([F, 1], f32, tag=f"shneg_{l}", name=f"shneg_{l}")
                nc.vector.tensor_scalar(shneg[:], mean, scal[:, 0:1], bet[l],
                                        op0=OP.mult, op1=OP.subtract)
                nc.vector._custom_dve(OP_BNRES, out=x_out, in0=agg[:],
                                      in1=x_in, s0=scal[:, 0:1], s1=shneg[:, 0:1])

            x1 = io.tile([F, NODES_PC], bf16, tag="x1")
            layer(1, xt16[:], x1[:])
            y32 = io.tile([F, NODES_PC], f32, tag="y32")
            layer(2, x1[:], y32[:])
            nc.sync.dma_start(yT.ap()[:, 0:256], y32[:][:, 0:256])
            nc.sync.dma_start(yT.ap()[:, 256:NODES_PC], y32[:][:, 256:NODES_PC])

    nc.compile()
    return nc


def _get_nc():
    if "nc" not in _CACHE:
        _CACHE["nc"] = _build_nc()
    return _CACHE["nc"]


def _canonical_edge_ok(src, dst):
    idx = np.arange(N_AGENTS)
    rows = np.repeat(idx, N_AGENTS)
    cols = np.tile(idx, N_AGENTS)
    m = rows != cols
    rows, cols = rows[m], cols[m]
    offs = (np.arange(N_SAMPLES) * N_AGENTS)[:, None]
    csrc = (rows[None, :] + offs).ravel().astype(np.int64)
    cdst = (cols[None, :] + offs).ravel().astype(np.int64)
    if src.shape != csrc.shape:
        return False
    key = np.sort(src.astype(np.int64) * N + dst.astype(np.int64))
    ckey = np.sort(csrc * N + cdst)
    return bool(np.array_equal(key, ckey))


def _numpy_fallback(gnn_in, centers, src, dst, Ws_all):
    def sig(x):
        return 1.0 / (1.0 + np.exp(-x))

    def sp(x):
        return np.log1p(np.exp(-np.abs(x))) + np.maximum(x, 0.0)

    x = gnn_in.astype(np.float64)
    e = (centers[dst] - centers[src]).astype(np.float64)
    for (Wf, bf, Wsm, bs, g, be) in Ws_all:
        z = np.concatenate([x[dst], x[src], e], axis=-1)
        msg = sig(z @ Wf.T + bf) * sp(z @ Wsm.T + bs)
        agg = np.zeros_like(x)
        np.add.at(agg, dst, msg)
        mean = agg.mean(0)
        var = agg.var(0)
        agg = (agg - mean) / np.sqrt(var + BN_EPS) * g + be
        x = np.maximum(agg + x, 0.0)
    return x.astype(np.float32)


def _host_weights(Wf, bf, Ws, bs):
    WaT = np.ascontiguousarray(Wf[:, :F].T)
    WbT = np.ascontiguousarray(Wf[:, F:2 * F].T)
    Wc = Wf[:, 2 * F:2 * F + EDIM].T
    z = np.zeros((1, F), np.float32)
    Wc3a = np.concatenate([Wc, bf[None, :]], 0)
    Wc3b = np.concatenate([-Wc, z], 0)
    VaT = np.ascontiguousarray(Ws[:, :F].T)
    VbT = np.ascontiguousarray(Ws[:, F:2 * F].T)
    Vc = Ws[:, 2 * F:2 * F + EDIM].T
    Vc3g = np.concatenate([Vc, bs[None, :]], 0)
    Vc3d = np.concatenate([-Vc, z], 0)
    return WaT, WbT, VaT, VbT, Wc3a, Wc3b, Vc3g, Vc3d


def _build_E16():
    E = np.zeros((F, N_AGENTS, N_AGENTS), np.float32)
    for t in range(N_AGENTS):
        E[t, t, :] = 1.0
        E[N_AGENTS + t, :, t] = 1.0
    E = E.reshape(F, PAIR)
    Ed = np.zeros((F, N_AGENTS), np.float32)
    for t in range(N_AGENTS):
        Ed[t, t] = 1.0
        Ed[N_AGENTS + t, t] = 1.0
    return E, Ed


def kernel(gnn_in, centers, src, dst,
           Wf1, bf1, Ws1, bs1, g1, be1,
           Wf2, bf2, Ws2, bs2, g2, be2,
           _trace=False, _tmpdir=None):
    import ml_dtypes
    b = ml_dtypes.bfloat16
    gnn_in = np.ascontiguousarray(np.asarray(gnn_in, np.float32))
    centers = np.ascontiguousarray(np.asarray(centers, np.float32))
    src = np.asarray(src, np.int32)
    dst = np.asarray(dst, np.int32)
    args = [np.asarray(a, np.float32) for a in
            (Wf1, bf1, Ws1, bs1, g1, be1, Wf2, bf2, Ws2, bs2, g2, be2)]
    (Wf1, bf1, Ws1, bs1, g1, be1, Wf2, bf2, Ws2, bs2, g2, be2) = args

    if not _canonical_edge_ok(src, dst):
        import sys
        print("kernel.py: edge index is not block-fully-connected; numpy fallback",
              file=sys.stderr)
        return _numpy_fallback(gnn_in, centers, src, dst,
                               [(Wf1, bf1, Ws1, bs1, g1, be1),
                                (Wf2, bf2, Ws2, bs2, g2, be2)])

    from concourse import bass_utils

    nc = _get_nc()

    E, Ed = _build_E16()
    w1 = _host_weights(Wf1, bf1, Ws1, bs1)
    w2 = _host_weights(Wf2, bf2, Ws2, bs2)
    wb = np.concatenate(list(w1[:4]) + list(w2[:4]) + [Ed], 1)
    wmap = {
        "wblob": np.ascontiguousarray(wb).astype(b),
        "Eblob": np.ascontiguousarray(E).astype(b),
        "gblob": np.ascontiguousarray(
            np.stack([g1, be1, g2, be2], 1).astype(np.float32)),
    }

    in_maps = []
    for k in range(N_CORES):
        sl = slice(k * NODES_PC, (k + 1) * NODES_PC)
        m = dict(wmap)
        m["xT16"] = np.ascontiguousarray(gnn_in[sl].T).astype(b)
        c3k = np.concatenate([centers[sl].T, np.ones((1, NODES_PC), np.float32)], 0)
        cb = np.concatenate([c3k] + list(w1[4:]) + list(w2[4:]), 1)
        m["c3blob"] = np.ascontiguousarray(cb).astype(b)
        c3b16 = c3k.astype(b).astype(np.float32)
        cps = []
        for w in (w1, w2):
            Wc3a = w[4].astype(b).astype(np.float32)
            Wc3b = w[5].astype(b).astype(np.float32)
            for c in range(S_PC):
                csl = c3b16[:, c * N_AGENTS:(c + 1) * N_AGENTS]
                cps.append(np.concatenate([csl.T @ Wc3a, csl.T @ Wc3b], 0))
        m["cpAT"] = np.ascontiguousarray(np.concatenate(cps, 1)).astype(b)
        in_maps.append(m)

    kw = {}
    if _trace:
        kw = dict(trace=True, tmpdir=_tmpdir)
    res = bass_utils.run_bass_kernel_spmd(nc, in_maps, core_ids=list(range(N_CORES)), **kw)

    out = np.empty((N, F), np.float32)
    for k in range(N_CORES):
        out[k * NODES_PC:(k + 1) * NODES_PC] = res.results[k]["yT"].T
    if _trace:
        _CACHE["last_res"] = res
    return out



# revision 56
# speedup vs baseline: 1.0467x; 1.0365x over previous
"""AgentGNN v2.1 (2x CGConv + BN + residual + ReLU) on 8 TRN2 NeuronCores.

Self-contained: FULL inputs -> shard 8 samples/core -> Bass kernel -> FULL out.

Engine mapping per layer (per core: 8 samples, pairwise = 8x[128,64,64]):
  PE:  per-sample transposed projections A_c[t,p] = [alpha^T; beta^T] (x and
       centers+bias parts as matmuls), then P1 = A_c @ E with E a fixed 0/1
       indicator [128, 4096+64]: P1[p,(i,j)] = alpha[p,i]+beta[p,j]; the
       extra 64 cols give the diagonal alpha_i+beta_i. Normal-orientation
       matmuls for gamma/delta.
  ACT: sigmoid straight from PSUM chunks -> S16 (bf16); softplus as
       ln(1+exp(g)*exp(d)-factorized); samples processed in two groups of 4
       so sigmoid<->ln table sets load 5x/layer instead of 16x.
  DVE: P2 outer-mult u = exp(gamma_i)*exp(delta_j) (broadcast tt bf16),
       fused mult+prefix-scan custom op on (S16,T16) with segment-end
       writes, per-segment diffs -> row sums; BN stats (sum/sumsq);
       fused BN-apply+residual+relu.
  BN stats via TWO AllReduces/layer (one per sample-group: the first hides
  under the second group's compute, only the second's latency is exposed).
  All weights/inputs packed into 4 DMA blobs to dodge per-transfer latency.
"""

import numpy as np

N_SAMPLES = 64
N_AGENTS = 64
N = N_SAMPLES * N_AGENTS          # 4096
F = 128
EDIM = 2
BN_EPS = 1e-5
N_CORES = 8
S_PC = N_SAMPLES // N_CORES       # 8 samples per core
NODES_PC = S_PC * N_AGENTS        # 512 nodes per core
PAIR = N_AGENTS * N_AGENTS        # 4096 pairwise per sample
G_SZ = 4                          # samples per ACT-table group
WCOLS = 8 * F + N_AGENTS          # wblob: 8 weight mats | Ed
CCOLS = NODES_PC + 8 * F          # c3blob: c3 | 8 small mats

_CACHE = {}


def _patch_ldw_opt():
    from concourse import bass_utils as BU

    if getattr(BU, "_ldw_patched", False):
        return
    orig = BU.run_command

    def rc(cmd, *a, **kw):
        if isinstance(cmd, list):
            cmd = ["--enable-ldw-opt=true" if c == "--enable-ldw-opt=false" else c
                   for c in cmd]
        return orig(cmd, *a, **kw)

    BU.run_command = rc
    BU._ldw_patched = True


def _register_custom_ops():
    import numpy as _np
    from concourse import dve_ops as D

    if getattr(D, "_agnn_ops", None):
        return D._agnn_ops
    from concourse.dve_spec import Spec, Src0, Src1, C0, C1, AluOp, scan, lower
    from concourse.dve_uop import DveOpSpec
    from concourse.dve_spec import relu as dve_relu

    def ref_mult_scan(in0, in1, s0, s1, imm2):
        prod = (in0.astype(_np.float32) * in1 - s0).astype(_np.float32)
        return _np.cumsum(prod.reshape(prod.shape[0], -1), 1).astype(
            _np.float32).reshape(in0.shape)

    def ref_diff_add(in0, in1, s0, s1, imm2):
        return (in0.astype(_np.float32) - in1 + s0).astype(_np.float32)

    def ref_bn_res(in0, in1, s0, s1, imm2):
        return _np.maximum(in0.astype(_np.float32) * s0 - s1 + in1, 0.0).astype(
            _np.float32)

    def make(name, spec, subdim):
        row = D._CUSTOM_DVE_ROW_BASE + len(D.OPS)
        D._SUB_OPCODE_FOR_NAME[name] = row
        shas = {}
        for ver in ("v3", "v4"):
            u = lower(spec, ver=ver)
            shas[ver] = DveOpSpec(name=name, opcode=row, uops=u, rd1_en=True).sha(ver)
        op = D.DveOp(name, spec, subdim=subdim, uops_sha=shas)
        D.OPS.append(op)
        D.CUSTOM_DVE_SPECS[name] = spec
        return op

    sc = Spec(body=scan(AluOp.ADD, Src0 * Src1 - C0), reference=ref_mult_scan)
    df = Spec(body=Src0 - Src1 + C0, reference=ref_diff_add)
    br = Spec(body=dve_relu(Src0 * C0 - C1 + Src1), reference=ref_bn_res)
    D._agnn_ops = (make("AGNN_MULT_CSCAN", sc, True),
                   make("AGNN_DIFF_ADD", df, False),
                   make("AGNN_BN_RES", br, False))
    return D._agnn_ops


def _patch_act_tables():
    from concourse import bacc, mybir, hw_specs

    if getattr(bacc, "_act_tables_patched", False):
        return
    AF = mybir.ActivationFunctionType
    orig = hw_specs.get_activation_tables

    def patched(arch):
        t = orig(arch)
        out = {}
        for name, s in t.items():
            s = set(s)
            if name == "exp_and_others":
                s.discard(AF.Exp)
            if name == "natural_log":
                s.discard(AF.Ln)
            out[name] = s
        return out

    bacc.get_activation_tables = patched
    bacc._act_tables_patched = True


def _build_nc():
    from concourse import bacc, mybir
    from concourse.tile import TileContext
    from concourse.tile_rust import add_dep_helper

    _patch_act_tables()
    OP_SCAN, OP_DIFF, OP_BNRES = _register_custom_ops()

    f32 = mybir.dt.float32
    bf16 = mybir.dt.bfloat16
    AF = mybir.ActivationFunctionType
    OP = mybir.AluOpType
    AX = mybir.AxisListType

    nc = bacc.Bacc(trn_type="TRN2", target_bir_lowering=False, debug=False,
                   num_devices=N_CORES)

    xT16 = nc.declare_dram_parameter("xT16", [F, NODES_PC], bf16, isOutput=False)
    wblob = nc.declare_dram_parameter("wblob", [F, WCOLS], bf16, isOutput=False)
    Eblob = nc.declare_dram_parameter("Eblob", [F, PAIR], bf16, isOutput=False)
    c3blob = nc.declare_dram_parameter("c3blob", [EDIM + 1, CCOLS], bf16, isOutput=False)
    gblob = nc.declare_dram_parameter("gblob", [F, 4], f32, isOutput=False)
    cpAT = nc.declare_dram_parameter("cpAT", [F, 2 * S_PC * F], bf16, isOutput=False)
    yT = nc.declare_dram_parameter("yT", [F, NODES_PC], f32, isOutput=True)

    cc_warm_in = nc.dram_tensor("cc_warm_in", [F, 2], f32)
    cc_warm_out = nc.dram_tensor("cc_warm_out", [F, 2], f32, addr_space="Shared")
    cc_in = {}
    cc_out = {}
    for l in (1, 2):
        for g in (0, 1):
            cc_in[(l, g)] = nc.dram_tensor(f"cc_in{l}{g}", [F, 2], f32)
            cc_out[(l, g)] = nc.dram_tensor(f"cc_out{l}{g}", [F, 2], f32,
                                            addr_space="Shared")

    with TileContext(nc) as tc:
        from contextlib import ExitStack
        with ExitStack() as ctx:
            io = ctx.enter_context(tc.tile_pool(name="io", bufs=1))
            node = ctx.enter_context(tc.tile_pool(name="node", bufs=1))
            pair = ctx.enter_context(tc.tile_pool(name="pair", bufs=1))
            psum = ctx.enter_context(tc.tile_pool(name="psum", bufs=1, space="PSUM"))
            small = ctx.enter_context(tc.tile_pool(name="small", bufs=1))

            # order: small tensors the first matmuls/exps need come first;
            # the big Eblob (1MB) and cpa stream in behind them.
            xt16 = io.tile([F, NODES_PC], bf16, tag="xt16")
            nc.sync.dma_start(xt16[:], xT16.ap()[:, :])
            c3b = io.tile([EDIM + 1, CCOLS], bf16, tag="c3b")
            nc.sync.dma_start(c3b[:], c3blob.ap()[:, :])
            wb = io.tile([F, WCOLS], bf16, tag="wb")
            nc.sync.dma_start(wb[:], wblob.ap()[:, :])
            gb = io.tile([F, 4], f32, tag="gb")
            nc.sync.dma_start(gb[:], gblob.ap()[:, :])
            cpa = io.tile([F, 2 * S_PC * F], bf16, tag="cpa")
            nc.sync.dma_start(cpa[:], cpAT.ap()[:, :])
            eb = io.tile([F, PAIR], bf16, tag="eb")
            nc.sync.dma_start(eb[:], Eblob.ap()[:, :])

            e16 = eb[:][:, :]
            ed16 = wb[:][:, 8 * F:8 * F + N_AGENTS]
            wslice = {}
            for li, l in enumerate((1, 2)):
                for wi, n in enumerate(("WaT", "WbT", "VaT", "VbT")):
                    c0 = (li * 4 + wi) * F
                    wslice[f"{n}{l}"] = wb[:][:, c0:c0 + F]
                for wi, n in enumerate(("Wc3a", "Wc3b", "Vc3g", "Vc3d")):
                    c0 = NODES_PC + (li * 4 + wi) * F
                    wslice[f"{n}{l}"] = c3b[:][:, c0:c0 + F]
            c3t = c3b[:][:, 0:NODES_PC]
            gam = {1: gb[:][:, 0:1], 2: gb[:][:, 2:3]}
            bet = {1: gb[:][:, 1:2], 2: gb[:][:, 3:4]}

            eps_t = small.tile([F, 1], f32, tag="eps")
            nc.vector.memset(eps_t[:], BN_EPS)
            zero1 = small.tile([F, 1], f32, tag="zero1")
            nc.vector.memset(zero1[:], 0.0)

            # one warm AR, triggered immediately: its doorbell absorbs the
            # ~60us CC-stream startup latency during the DMA-in/head phase
            nc.gpsimd.collective_compute(
                "AllReduce", mybir.AluOpType.add,
                replica_groups=[list(range(N_CORES))],
                ins=[cc_warm_in.ap().opt()], outs=[cc_warm_out.ap().opt()])

            act_chain = []

            def act(*args, **kw):
                i = nc.scalar.activation(*args, **kw)
                if act_chain:
                    add_dep_helper(i.ins, act_chain[-1].ins, reason="act order")
                act_chain.append(i)
                return i

            # persistent tiles
            Sall = pair.tile([F, S_PC * PAIR], bf16, tag="Sall")
            Uall = pair.tile([F, S_PC * PAIR], bf16, tag="Uall")
            S16 = [Sall[:][:, c * PAIR:(c + 1) * PAIR] for c in range(S_PC)]
            U16 = [Uall[:][:, c * PAIR:(c + 1) * PAIR] for c in range(S_PC)]
            A16 = [node.tile([F, F], bf16, tag=f"A{c}", name=f"A16_{c}")
                   for c in range(2)]
            eg = node.tile([F, NODES_PC], bf16, tag="eg")
            edt = node.tile([F, NODES_PC], bf16, tag="edt")
            d1s = node.tile([F, NODES_PC], bf16, tag="d1s")
            d2s = node.tile([F, NODES_PC], bf16, tag="d2s")
            dm = node.tile([F, NODES_PC], bf16, tag="dm")
            agg = node.tile([F, NODES_PC], f32, tag="agg")
            sq = node.tile([F, NODES_PC], f32, tag="sq")
            pref = node.tile([F, S_PC, N_AGENTS + 1], f32, tag="pref")

            psA = [psum.tile([F, 192], f32, tag=f"psA{b}", name=f"psA{b}")
                   for b in range(2)]
            psS = [psum.tile([F, 1024], f32, tag=f"psS{b}", name=f"psS{b}")
                   for b in range(3)]

            def pe_warm(n, bufs=(0, 1, 2)):
                # keep PE_HAM's activity window busy so the clock stays 8/8
                for i in range(n):
                    pst = psS[bufs[i % len(bufs)]]
                    nc.tensor.matmul(pst[:][:, 0:512], wslice["WaT1"],
                                     wb[:][:, 0:512], start=True, stop=True)

            def layer(l, x_in, x_out):
                # gamma/delta normal-orientation matmuls into psS[0] + exps
                nc.tensor.matmul(psS[0][:][:, 0:512], wslice[f"VaT{l}"], x_in,
                                 start=True, stop=False)
                nc.tensor.matmul(psS[0][:][:, 0:512], wslice[f"Vc3g{l}"], c3t,
                                 start=False, stop=True)
                nc.tensor.matmul(psS[0][:][:, 512:1024], wslice[f"VbT{l}"], x_in,
                                 start=True, stop=False)
                nc.tensor.matmul(psS[0][:][:, 512:1024], wslice[f"Vc3d{l}"], c3t,
                                 start=False, stop=True)
                act(eg[:], psS[0][:][:, 0:512], AF.Exp)
                act(edt[:], psS[0][:][:, 512:1024], AF.Exp)
                # dm = eg*ed (node level, feeds d2s = ln1p) — Pool engine
                dm_op = nc.gpsimd.tensor_tensor(dm[:], eg[:], edt[:], op=OP.mult)
                # all 8 outer-product u-mults on the (otherwise idle) Pool
                # engine, emitted early so U16[c] is ready before ACT's Ln
                def u_mult(c, eng):
                    sl_ = slice(c * N_AGENTS, (c + 1) * N_AGENTS)
                    g_bc = eg[:, sl_].broadcast_to([F, N_AGENTS, N_AGENTS])
                    d_bc = edt[:, sl_].rearrange("p (o j) -> p o j", o=1) \
                        .broadcast_to([F, N_AGENTS, N_AGENTS])
                    return eng.tensor_tensor(
                        U16[c].rearrange("p (i j) -> p i j", i=N_AGENTS),
                        g_bc, d_bc, op=OP.mult)

                u_last = dm_op

                s2 = {g: small.tile([F, 2], f32, tag=f"s2_{l}{g}",
                                    name=f"s2_{l}{g}") for g in (0, 1)}
                gst = {g: small.tile([F, 2], f32, tag=f"gst_{l}{g}",
                                     name=f"gst_{l}{g}") for g in (0, 1)}
                din = {}

                nc.vector.memset(pref[:, :, 0:1], 0.0)

                def sample_block(c, first, last):
                    # PE projections + E-matmul chunks + sigmoids for sample c
                    sl = slice(c * N_AGENTS, (c + 1) * N_AGENTS)
                    pa = psA[c % 2]
                    cp0 = (l - 1) * S_PC * F + c * F
                    nc.tensor.matmul(pa[:][0:64, 0:128], x_in[:, sl],
                                     wslice[f"WaT{l}"], start=True, stop=True)
                    nc.tensor.matmul(pa[:][64:128, 0:128], x_in[:, sl],
                                     wslice[f"WbT{l}"], start=True, stop=True)
                    a16 = A16[c % 2]
                    nc.vector.tensor_tensor(a16[:], pa[:][:, 0:128],
                                            cpa[:][:, cp0:cp0 + F], op=OP.add)
                    # DVE work for the PREVIOUS sample goes here (after the
                    # add, so E-matmuls never wait a long DVE op for a16):
                    # group 0 interleaves u-mults, group 1 interleaves scans
                    if not first:
                        if c <= G_SZ:
                            u_mult(c - 1, nc.vector)
                        else:
                            scan(c - 1)
                    for q in range(4):
                        pst = psS[q % 3]
                        for h in range(2):
                            col = q * 1024 + h * 512
                            nc.tensor.matmul(
                                pst[:][:, h * 512:(h + 1) * 512], a16[:],
                                e16[:, col:col + 512], start=True, stop=True)
                        act(S16[c][:, q * 1024:(q + 1) * 1024], pst[:],
                            AF.Sigmoid)
                    nc.tensor.matmul(pa[:][:, 128:192], a16[:], ed16,
                                     start=True, stop=True)
                    act(d1s[:][:, sl], pa[:][:, 128:192], AF.Sigmoid)
                    if last:
                        if c < G_SZ:
                            u_mult(c, nc.vector)
                        else:
                            scan(c)

                def scan(c):
                    scan_out = pref[:, c, 1:1 + N_AGENTS] \
                        .rearrange("p (i o) -> p i o", o=1) \
                        .broadcast_to([F, N_AGENTS, N_AGENTS])
                    nc.vector._custom_dve(
                        OP_SCAN, out=scan_out,
                        in0=S16[c].rearrange("p (i j) -> p i j", i=N_AGENTS),
                        in1=U16[c].rearrange("p (i j) -> p i j", i=N_AGENTS),
                        s0=zero1[:, 0:1])

                def group_stats(g):
                    gsl = slice(g * G_SZ * N_AGENTS, (g + 1) * G_SZ * N_AGENTS)
                    gs = slice(g * G_SZ, (g + 1) * G_SZ)
                    nc.vector._custom_dve(
                        OP_DIFF,
                        out=agg[:, gsl].rearrange("p (c i) -> p c i", c=G_SZ),
                        in0=pref[:, gs, 1:1 + N_AGENTS],
                        in1=pref[:, gs, 0:N_AGENTS],
                        s0=zero1[:, 0:1])
                    return gsl

                def stats_and_ar(g):
                    gsl = group_stats(g)
                    # self-msg subtract + BN partial stats for this group (DVE)
                    nc.vector.tensor_tensor(dm[:, gsl], d1s[:, gsl], d2s[:, gsl],
                                            op=OP.mult)
                    nc.vector.tensor_tensor(agg[:, gsl], agg[:, gsl], dm[:, gsl],
                                            op=OP.subtract)
                    nc.vector.tensor_reduce(s2[g][:, 0:1], agg[:, gsl],
                                            axis=AX.X, op=OP.add)
                    nc.vector.tensor_tensor(sq[:, gsl], agg[:, gsl], agg[:, gsl],
                                            op=OP.mult)
                    nc.vector.tensor_reduce(s2[g][:, 1:2], sq[:, gsl],
                                            axis=AX.X, op=OP.add)
                    dsum = nc.sync.dma_start(cc_in[(l, g)].ap()[:, :], s2[g][:])
                    ar = nc.gpsimd.collective_compute(
                        "AllReduce", mybir.AluOpType.add,
                        replica_groups=[list(range(N_CORES))],
                        ins=[cc_in[(l, g)].ap().opt()],
                        outs=[cc_out[(l, g)].ap().opt()])
                    add_dep_helper(ar.ins, dsum.ins, reason="cc reads cc_in")
                    # keep the AR's Pool-queue doorbell BEHIND all u-mults
                    # (head-of-line blocking otherwise stalls them on CC)
                    add_dep_helper(ar.ins, u_last.ins, reason="doorbell after u-mults")
                    din[g] = nc.sync.dma_start(gst[g][:], cc_out[(l, g)].ap()[:, :])
                    add_dep_helper(din[g].ins, ar.ins, reason="dma reads cc_out")


                # ---- phase 1: group-0 samples (sigmoid table), u-mults 0-3
                # interleaved on DVE; u4-7 emitted right after (they only
                # need eg/edt and must be done before the batched Ln section)
                for c in range(G_SZ):
                    sample_block(c, c == 0, c == G_SZ - 1)
                for c in range(G_SZ, S_PC):
                    u_mult(c, nc.vector)
                # ---- phase 2: ONE ln-table section for d2s + ALL 8 U16 Lns;
                # group-0 scans interleave behind their Lns on DVE
                act(d2s[:], dm[:], AF.Ln, bias=1.0)
                for c in range(G_SZ):
                    act(U16[c], U16[c], AF.Ln, bias=1.0)
                    scan(c)
                stats_and_ar(0)
                for c in range(G_SZ, S_PC):
                    act(U16[c], U16[c], AF.Ln, bias=1.0)
                # ---- phase 3: group-1 samples (sigmoid table) with their
                # scans interleaved (U16 already Ln'd); then stats
                for c in range(G_SZ, S_PC):
                    sample_block(c, c == G_SZ, c == S_PC - 1)

                stats_and_ar(1)

                # ---- BN apply + residual + relu ----
                gall = small.tile([F, 2], f32, tag=f"gall_{l}", name=f"gall_{l}")
                nc.vector.tensor_tensor(gall[:], gst[0][:], gst[1][:], op=OP.add)
                me2 = small.tile([F, 2], f32, tag=f"me2_{l}", name=f"me2_{l}")
                nc.vector.tensor_scalar(me2[:], gall[:], 1.0 / N, None, op0=OP.mult)
                mean, ex2 = me2[:, 0:1], me2[:, 1:2]
                var = small.tile([F, 1], f32, tag=f"var_{l}", name=f"var_{l}")
                nc.vector.tensor_tensor(var[:], mean, mean, op=OP.mult)
                nc.vector.tensor_tensor(var[:], ex2, var[:], op=OP.subtract)
                lnv = small.tile([F, 1], f32, tag=f"lnv_{l}", name=f"lnv_{l}")
                act(lnv[:], var[:], AF.Ln, bias=eps_t[:])
                rstd = small.tile([F, 1], f32, tag=f"rstd_{l}", name=f"rstd_{l}")
                act(rstd[:], lnv[:], AF.Exp, bias=0.0, scale=-0.5)
                scal = small.tile([F, 1], f32, tag=f"scal_{l}", name=f"scal_{l}")
                nc.vector.tensor_tensor(scal[:], rstd[:], gam[l], op=OP.mult)
                shneg = small.tile([F, 1], f32, tag=f"shneg_{l}", name=f"shneg_{l}")
                nc.vector.tensor_scalar(shneg[:], mean, scal[:, 0:1], bet[l],
                                        op0=OP.mult, op1=OP.subtract)
                nc.vector._custom_dve(OP_BNRES, out=x_out, in0=agg[:],
                                      in1=x_in, s0=scal[:, 0:1], s1=shneg[:, 0:1])

            x1 = io.tile([F, NODES_PC], bf16, tag="x1")
            layer(1, xt16[:], x1[:])
            y32 = io.tile([F, NODES_PC], f32, tag="y32")
            layer(2, x1[:], y32[:])
            nc.sync.dma_start(yT.ap()[:, 0:256], y32[:][:, 0:256])
            nc.sync.dma_start(yT.ap()[:, 256:NODES_PC], y32[:][:, 256:NODES_PC])

    nc.compile()
    return nc


def _get_nc():
    if "nc" not in _CACHE:
        _CACHE["nc"] = _build_nc()
    return _CACHE["nc"]


def _canonical_edge_ok(src, dst):
    idx = np.arange(N_AGENTS)
    rows = np.repeat(idx, N_AGENTS)
    cols = np.tile(idx, N_AGENTS)
    m = rows != cols
    rows, cols = rows[m], cols[m]
    offs = (np.arange(N_SAMPLES) * N_AGENTS)[:, None]
    csrc = (rows[None, :] + offs).ravel().astype(np.int64)
    cdst = (cols[None, :] + offs).ravel().astype(np.int64)
    if src.shape != csrc.shape:
        return False
    key = np.sort(src.astype(np.int64) * N + dst.astype(np.int64))
    ckey = np.sort(csrc * N + cdst)
    return bool(np.array_equal(key, ckey))


def _numpy_fallback(gnn_in, centers, src, dst, Ws_all):
    def sig(x):
        return 1.0 / (1.0 + np.exp(-x))

    def sp(x):
        return np.log1p(np.exp(-np.abs(x))) + np.maximum(x, 0.0)

    x = gnn_in.astype(np.float64)
    e = (centers[dst] - centers[src]).astype(np.float64)
    for (Wf, bf, Wsm, bs, g, be) in Ws_all:
        z = np.concatenate([x[dst], x[src], e], axis=-1)
        msg = sig(z @ Wf.T + bf) * sp(z @ Wsm.T + bs)
        agg = np.zeros_like(x)
        np.add.at(agg, dst, msg)
        mean = agg.mean(0)
        var = agg.var(0)
        agg = (agg - mean) / np.sqrt(var + BN_EPS) * g + be
        x = np.maximum(agg + x, 0.0)
    return x.astype(np.float32)


def _host_weights(Wf, bf, Ws, bs):
    WaT = np.ascontiguousarray(Wf[:, :F].T)
    WbT = np.ascontiguousarray(Wf[:, F:2 * F].T)
    Wc = Wf[:, 2 * F:2 * F + EDIM].T
    z = np.zeros((1, F), np.float32)
    Wc3a = np.concatenate([Wc, bf[None, :]], 0)
    Wc3b = np.concatenate([-Wc, z], 0)
    VaT = np.ascontiguousarray(Ws[:, :F].T)
    VbT = np.ascontiguousarray(Ws[:, F:2 * F].T)
    Vc = Ws[:, 2 * F:2 * F + EDIM].T
    Vc3g = np.concatenate([Vc, bs[None, :]], 0)
    Vc3d = np.concatenate([-Vc, z], 0)
    return WaT, WbT, VaT, VbT, Wc3a, Wc3b, Vc3g, Vc3d


def _build_E16():
    E = np.zeros((F, N_AGENTS, N_AGENTS), np.float32)
    for t in range(N_AGENTS):
        E[t, t, :] = 1.0
        E[N_AGENTS + t, :, t] = 1.0
    E = E.reshape(F, PAIR)
    Ed = np.zeros((F, N_AGENTS), np.float32)
    for t in range(N_AGENTS):
        Ed[t, t] = 1.0
        Ed[N_AGENTS + t, t] = 1.0
    return E, Ed


def kernel(gnn_in, centers, src, dst,
           Wf1, bf1, Ws1, bs1, g1, be1,
           Wf2, bf2, Ws2, bs2, g2, be2,
           _trace=False, _tmpdir=None):
    import ml_dtypes
    b = ml_dtypes.bfloat16
    gnn_in = np.ascontiguousarray(np.asarray(gnn_in, np.float32))
    centers = np.ascontiguousarray(np.asarray(centers, np.float32))
    src = np.asarray(src, np.int32)
    dst = np.asarray(dst, np.int32)
    args = [np.asarray(a, np.float32) for a in
            (Wf1, bf1, Ws1, bs1, g1, be1, Wf2, bf2, Ws2, bs2, g2, be2)]
    (Wf1, bf1, Ws1, bs1, g1, be1, Wf2, bf2, Ws2, bs2, g2, be2) = args

    if not _canonical_edge_ok(src, dst):
        import sys
        print("kernel.py: edge index is not block-fully-connected; numpy fallback",
              file=sys.stderr)
        return _numpy_fallback(gnn_in, centers, src, dst,
                               [(Wf1, bf1, Ws1, bs1, g1, be1),
                                (Wf2, bf2, Ws2, bs2, g2, be2)])

    from concourse import bass_utils

    nc = _get_nc()

    E, Ed = _build_E16()
    w1 = _host_weights(Wf1, bf1, Ws1, bs1)
    w2 = _host_weights(Wf2, bf2, Ws2, bs2)
    wb = np.concatenate(list(w1[:4]) + list(w2[:4]) + [Ed], 1)
    wmap = {
        "wblob": np.ascontiguousarray(wb).astype(b),
        "Eblob": np.ascontiguousarray(E).astype(b),
        "gblob": np.ascontiguousarray(
            np.stack([g1, be1, g2, be2], 1).astype(np.float32)),
    }

    in_maps = []
    for k in range(N_CORES):
        sl = slice(k * NODES_PC, (k + 1) * NODES_PC)
        m = dict(wmap)
        m["xT16"] = np.ascontiguousarray(gnn_in[sl].T).astype(b)
        c3k = np.concatenate([centers[sl].T, np.ones((1, NODES_PC), np.float32)], 0)
        cb = np.concatenate([c3k] + list(w1[4:]) + list(w2[4:]), 1)
        m["c3blob"] = np.ascontiguousarray(cb).astype(b)
        c3b16 = c3k.astype(b).astype(np.float32)
        cps = []
        for w in (w1, w2):
            Wc3a = w[4].astype(b).astype(np.float32)
            Wc3b = w[5].astype(b).astype(np.float32)
            for c in range(S_PC):
                csl = c3b16[:, c * N_AGENTS:(c + 1) * N_AGENTS]
                cps.append(np.concatenate([csl.T @ Wc3a, csl.T @ Wc3b], 0))
        m["cpAT"] = np.ascontiguousarray(np.concatenate(cps, 1)).astype(b)
        in_maps.append(m)

    kw = {}
    if _trace:
        kw = dict(trace=True, tmpdir=_tmpdir)
    res = bass_utils.run_bass_kernel_spmd(nc, in_maps, core_ids=list(range(N_CORES)), **kw)

    out = np.empty((N, F), np.float32)
    for k in range(N_CORES):
        out[k * NODES_PC:(k + 1) * NODES_PC] = res.results[k]["yT"].T
    if _trace:
        _CACHE["last_res"] = res
    return out

